# revision 19
# baseline (speedup 1.0000x reference)
"""Trainium2 Bass kernel for nn_Bottleneck (dynamic-routing bottleneck block).

Data-parallel over batch: 32 samples -> 8 NeuronCores x 4 samples.

Per-core pipeline (per sample):
  x -> [base]  avgpool3x3s2 -> 1x1 -> 3x3 -> 1x1 (bf16 matmuls, N=196)
    -> [mask]  grouped 3x3 conv as channel-contraction (M=72) + tap-shift
               scatter + row-tiled tap-sum matmuls, adaptive pool, grouped fc,
               hard mask = (l1 + db > l0)            (float32r + fp32 fc)
    -> [refine] grouped 1x1 -> 3x3 -> 1x1 with mask gating (float32r, N=392)
  out = relu(refine + upsample(base) + x)

3x3 convs = 9 shifted-AP taps accumulating in PSUM over zero-padded SBUF
layouts.  BN is folded into conv weights/biases in numpy.  Mask multiplies
use mask in {0,1} exactness: t1 = m * relu(bn1(conv1r(x))) etc.
"""
import sys

for _p in ("/opt/trn_rl_repo",):
    if _p not in sys.path:
        sys.path.insert(0, _p)

import numpy as np
import ml_dtypes

import concourse.bass as bass
import concourse.bacc as bacc
import concourse.tile as tile
import concourse.mybir as mybir
from concourse.bass_utils import run_bass_kernel_spmd

F32 = mybir.dt.float32
F32R = mybir.dt.float32r
BF16 = mybir.dt.bfloat16
U32 = mybir.dt.uint32
RELU = mybir.ActivationFunctionType.Relu
COPY = mybir.ActivationFunctionType.Copy
IDENT = mybir.ActivationFunctionType.Identity
ADD = mybir.AluOpType.add
MULT = mybir.AluOpType.mult
ISGT = mybir.AluOpType.is_gt

B, C, H, W = 32, 1024, 28, 28
G, MS, PL, MID = 2, 7, 1024, 256
EPS = 1e-5
NCORES = 8
SPC = B // NCORES      # samples per core
NCH = C // 128         # 8 channel chunks
BF = ml_dtypes.bfloat16

_cache = {}


def _fold(w, bn):
    """Fold eval-mode BN into conv weight/bias. w [O,I,kh,kw], bn [4,O]."""
    g, b, m, v = np.asarray(bn, dtype=np.float64)
    s = g / np.sqrt(v + EPS)
    w = np.asarray(w, dtype=np.float64) * s[:, None, None, None]
    return w, (b - m * s)


def _prep_weights(mg_conv_w, mg_bn, mg_fc_w, mg_fc_b,
                  c1b_w, bn1b, c2b_w, bn2b, c3b_w, bn3b,
                  c1r_w, bn1r, c2r_w, bn2r, c3r_w, bn3r):
    W1b, b1b = _fold(c1b_w, bn1b)
    W1b /= 9.0                       # avgpool /9 folded in
    W2b, b2b = _fold(c2b_w, bn2b)
    W3b, b3b = _fold(c3b_w, bn3b)
    W1r, b1r = _fold(c1r_w, bn1r)
    W2r, b2r = _fold(c2r_w, bn2r)
    W3r, b3r = _fold(c3r_w, bn3r)
    Wmg, bmg = _fold(mg_conv_w, mg_bn)

    d = {}
    # base branch (bf16 lhsT layouts, partition-major)
    d["w1b"] = W1b.reshape(256, 8, 128).transpose(2, 1, 0).astype(BF)
    d["w2b"] = (W2b.reshape(256, 2, 128, 3, 3)
                .reshape(256, 2, 128, 9).transpose(2, 1, 3, 0).astype(BF))
    d["w3b"] = W3b.reshape(1024, 2, 128).transpose(2, 1, 0).astype(BF)
    d["b1b"] = np.asarray(b1b, np.float32).reshape(2, 128).T.copy()
    d["b2b"] = np.asarray(b2b, np.float32).reshape(2, 128).T.copy()
    d["b3b"] = np.asarray(b3b, np.float32).reshape(8, 128).T.copy()
    # refine branch (fp32 storage, f32r matmuls)
    d["w1r"] = (W1r.reshape(2, 128, 4, 128).transpose(3, 2, 0, 1)
                .astype(np.float32))
    d["w2r"] = (W2r.reshape(2, 128, 128, 9).transpose(2, 0, 3, 1)
                .astype(np.float32))
    d["w3r"] = W3r.reshape(2, 512, 128).transpose(2, 0, 1).astype(np.float32)
    d["b1r"] = np.asarray(b1r, np.float32).reshape(2, 128).T.copy()
    d["b2r"] = np.asarray(b2r, np.float32).reshape(2, 128).T.copy()
    d["b3r"] = np.asarray(b3r, np.float32).reshape(8, 128).T.copy()
    # mask generator: wmg [128, 8, 72], col = tap*8 + oc
    wmg = np.zeros((128, 8, 72), np.float64)
    a = Wmg.reshape(8, 4, 128, 9)          # [oc, kc_in_group, p, tap]
    for oc in range(8):
        gg = oc // 4
        for kc in range(4):
            for tp in range(9):
                wmg[:, 4 * gg + kc, tp * 8 + oc] = a[oc, kc, :, tp]
    d["wmg"] = wmg.astype(np.float32)
    d["bmg"] = np.asarray(bmg, np.float32).reshape(8, 1)
    d["i8"] = np.eye(8, dtype=np.float32)
    # grouped fc: [8, 4] fp32, col order (i,g) = (j//2, j%2); /16 (adaptive
    # pool mean) folded in
    wfc = np.zeros((8, 2), np.float64)
    Wf = np.asarray(mg_fc_w, np.float64).reshape(4, 4)   # [oc, ic_local]
    for g_ in range(2):
        # col g = (l1 - l0) weights for group g
        wfc[4 * g_:4 * g_ + 4, g_] = (Wf[g_ * 2 + 1] - Wf[g_ * 2]) / 16.0
    d["wfc"] = wfc.astype(np.float32)
    fb = np.asarray(mg_fc_b, np.float64)
    d["dfc"] = np.array([[fb[1] - fb[0]], [fb[3] - fb[2]]], np.float32)
    return d


def _build_nc():
    nc = bacc.Bacc(None, target_bir_lowering=False)

    x_d = nc.declare_dram_parameter("x", [SPC, C, H, W], F32R, isOutput=False)
    wd = {}
    for nm, shape, dt in [
        ("w1b", [128, 8, 256], BF16), ("w2b", [128, 2, 9, 256], BF16),
        ("w3b", [128, 2, 1024], BF16),
        ("b1b", [128, 2], F32), ("b2b", [128, 2], F32), ("b3b", [128, 8], F32),
        ("w1r", [128, 4, 2, 128], F32R), ("w2r", [128, 2, 9, 128], F32R),
        ("w3r", [128, 2, 512], F32R),
        ("b1r", [128, 2], F32), ("b2r", [128, 2], F32), ("b3r", [128, 8], F32),
        ("wmg", [128, 8, 72], F32R), ("bmg", [8, 1], F32),
        ("i8", [8, 8], F32R), ("wfc", [8, 2], F32), ("dfc", [2, 1], F32),
    ]:
        wd[nm] = nc.declare_dram_parameter(nm, shape, dt, isOutput=False)
    out_d = nc.declare_dram_parameter("out", [SPC, C, H, W], F32, isOutput=True)
    mask_d = nc.declare_dram_parameter("mask", [SPC, G, MS, MS], F32R,
                                       isOutput=True)

    with tile.TileContext(nc) as tc:
        with tc.tile_pool(name="sb", bufs=1) as sb, \
             tc.tile_pool(name="ps", bufs=1, space="PSUM") as ps, \
             tc.tile_pool(name="dramp", bufs=2, space="DRAM") as dramp:
            _body(nc, tc, sb, ps, dramp, x_d, wd, out_d, mask_d)
    nc.finalize()
    return nc


def _body(nc, tc, sb, ps, dramp, x_d, wd, out_d, mask_d):
    r32 = lambda ap: ap

    # ---- resident weights ----
    w = {}
    for nm, dt in [("w1b", BF16), ("w2b", BF16), ("w3b", BF16),
                   ("b1b", F32), ("b2b", F32), ("b3b", F32),
                   ("w1r", F32R), ("w2r", F32R), ("w3r", F32R),
                   ("b1r", F32), ("b2r", F32), ("b3r", F32),
                   ("wmg", F32R), ("bmg", F32), ("i8", F32R), ("wfc", F32),
                   ("dfc", F32)]:
        t = sb.tile(list(wd[nm].shape), dt, tag=f"w_{nm}")
        nc.sync.dma_start(out=t, in_=wd[nm][...])
        w[nm] = t

    # ---- persistent padded buffers (borders stay zero across samples) ----
    x_pads = []
    for i in range(2):
        t = sb.tile([128, NCH, 30, 30], F32R, tag=f"xpad{i}")
        nc.gpsimd.memset(t.bitcast(U32), 0)
        x_pads.append(t)
    xb1 = sb.tile([128, 2, 16, 16], BF16, tag="xb1")
    nc.gpsimd.memset(xb1, 0.0)
    t_pad72 = sb.tile([72, 30, 30], F32R, tag="tpad72")
    nc.gpsimd.memset(t_pad72.bitcast(U32), 0)
    t1_pad = sb.tile([128, 2, 30, 30], F32R, tag="t1pad")
    nc.gpsimd.memset(t1_pad.bitcast(U32), 0)

    def dma_in(s):
        xp = x_pads[s % 2]
        for k in range(NCH):
            nc.sync.dma_start(
                out=xp[:, k, 1:29, 1:29],
                in_=x_d[s, k * 128:(k + 1) * 128])

    taps = [(ti // 3, ti % 3) for ti in range(9)]

    def compute(s):
        xp = x_pads[s % 2]

        # ================= base branch (bf16) =================
        # avgpool 3x3 s2 p1: separable sums (÷9 folded into w1b)
        A = sb.tile([128, NCH, 30, 14], BF16, tag="poolA", bufs=1)
        va = xp[:, :, :, 0:28:2]
        vb = xp[:, :, :, 1:29:2]
        vc = xp[:, :, :, 2:30:2]
        nc.vector.tensor_tensor(out=A, in0=va, in1=vb, op=ADD)
        nc.vector.tensor_tensor(out=A, in0=A, in1=vc, op=ADD)
        xb0 = sb.tile([128, NCH, 196], BF16, tag="xb0", bufs=1)
        xb0v = xb0.rearrange("p k (a b) -> p k a b", a=14)
        r0 = A[:, :, 0:28:2, :]
        r1 = A[:, :, 1:29:2, :]
        r2 = A[:, :, 2:30:2, :]
        nc.vector.tensor_tensor(out=xb0v, in0=r0, in1=r1, op=ADD)
        nc.vector.tensor_tensor(out=xb0v, in0=xb0v, in1=r2, op=ADD)

        # conv1b 1x1 1024->256 @14x14
        for m in range(2):
            pt = ps.tile([128, 512], F32, tag="p1", bufs=2)
            for k in range(NCH):
                nc.tensor.matmul(pt[:, :196],
                                 w["w1b"][:, k, m * 128:(m + 1) * 128],
                                 xb0[:, k, :],
                                 start=(k == 0), stop=(k == NCH - 1))
            nc.scalar.activation(
                out=xb1[:, m, 1:15, 1:15],
                in_=pt[:, :196].rearrange("p (a b) -> p a b", a=14),
                func=RELU, bias=w["b1b"][:, m:m + 1], scale=1.0)

        # conv2b 3x3 256->256 @14x14 (9 taps over padded xb1)
        xb2 = sb.tile([128, 2, 196], BF16, tag="xb2", bufs=2)
        for m in range(2):
            pt = ps.tile([128, 512], F32, tag="p1", bufs=2)
            first = True
            for k in range(2):
                for ti, (dy, dx) in enumerate(taps):
                    nc.tensor.matmul(
                        pt[:, :196],
                        w["w2b"][:, k, ti, m * 128:(m + 1) * 128],
                        xb1[:, k, dy:dy + 14, dx:dx + 14],
                        start=first, stop=(k == 1 and ti == 8))
                    first = False
            nc.scalar.activation(out=xb2[:, m, :], in_=pt[:, :196],
                                 func=RELU, bias=w["b2b"][:, m:m + 1],
                                 scale=1.0)

        # conv3b 1x1 256->1024 @14x14, bn only -> xb3 fp32
        xb3 = sb.tile([128, NCH, 196], F32, tag="xb3", bufs=1)
        for mc in range(NCH):
            pt = ps.tile([128, 512], F32, tag="p1", bufs=2)
            for k in range(2):
                nc.tensor.matmul(pt[:, :196],
                                 w["w3b"][:, k, mc * 128:(mc + 1) * 128],
                                 xb2[:, k, :],
                                 start=(k == 0), stop=(k == 1))
            nc.scalar.activation(out=xb3[:, mc, :], in_=pt[:, :196],
                                 func=IDENT, bias=w["b3b"][:, mc:mc + 1],
                                 scale=1.0)

        # ================= mask generation (f32r) =================
        # stage 1: per-tap channel contraction, M = 72 = (tap, oc)
        pt_t = ps.tile([72, 2, 512], F32, tag="p2", bufs=3)
        for h in range(2):
            for k in range(NCH):
                nc.tensor.matmul(
                    pt_t[:, h, :392],
                    r32(w["wmg"][:, k, :]),
                    r32(xp[:, k, 1 + h * 14:15 + h * 14, 1:29]),
                    start=(k == 0), stop=(k == NCH - 1))
        # copy into padded [72, 30, 30]
        nc.scalar.activation(
            out=t_pad72[:, 1:29, 1:29].rearrange("p (h a) b -> p h a b", h=2),
            in_=pt_t[:, :, :392].rearrange("p h (a b) -> p h a b", a=14),
            func=COPY)
        # scatter taps to partitions 0..7 with the shift baked into free dim
        t_sb = sb.tile([8, 9, 784], F32R, tag="tsb", bufs=1)
        for ti, (dy, dx) in enumerate(taps):
            nc.sync.dma_start(
                out=t_sb[:, ti, :].rearrange("p (a b) -> p a b", a=28),
                in_=t_pad72[8 * ti:8 * ti + 8, dy:dy + 28, dx:dx + 28])
        # stage 2: tap sum via plain K=8 matmuls accumulating in PSUM
        pt_g = ps.tile([8, 2, 512], F32, tag="p2", bufs=3)
        for h in range(2):
            for ti in range(9):
                nc.tensor.matmul(
                    pt_g[:, h, :392],
                    r32(w["i8"]),
                    r32(t_sb[:, ti, :].rearrange(
                        "p (h2 a b) -> p h2 a b", h2=2, a=14)[:, h]),
                    start=(ti == 0), stop=(ti == 8))
        gte = sb.tile([8, 784], F32, tag="gte", bufs=1)
        nc.scalar.activation(out=gte.rearrange("p (h f) -> p h f", h=2),
                             in_=pt_g[:, :, :392], func=RELU,
                             bias=w["bmg"], scale=1.0)
        # adaptive avg pool 28x28 -> 7x7 (sums; /16 folded into wfc)
        ga = sb.tile([8, 196], F32, tag="ga", bufs=2)
        gav = ga.rearrange("p (a b) -> p a b", a=28)
        gv = gte.rearrange("p (a b c) -> p a b c", a=28, b=7)
        nc.vector.tensor_tensor(out=gav, in0=gv[:, :, :, 0], in1=gv[:, :, :, 1],
                                op=ADD)
        nc.vector.tensor_tensor(out=gav, in0=gav, in1=gv[:, :, :, 2], op=ADD)
        nc.vector.tensor_tensor(out=gav, in0=gav, in1=gv[:, :, :, 3], op=ADD)
        gb = sb.tile([8, 49], F32, tag="gb", bufs=2)
        gbv = gb.rearrange("p (a b) -> p a b", a=7)
        gaq = ga.rearrange("p (a r b) -> p a r b", a=7, r=4)
        nc.vector.tensor_tensor(out=gbv, in0=gaq[:, :, 0], in1=gaq[:, :, 1],
                                op=ADD)
        nc.vector.tensor_tensor(out=gbv, in0=gbv, in1=gaq[:, :, 2], op=ADD)
        nc.vector.tensor_tensor(out=gbv, in0=gbv, in1=gaq[:, :, 3], op=ADD)
        # grouped fc (fp32 matmul, exact) + hard mask
        pt_l = ps.tile([8, 512], F32, tag="p1", bufs=2)
        nc.tensor.matmul(pt_l[0:2, :49], w["wfc"], gb, start=True, stop=True)
        # mask rows at partitions 0 and 32 so the PE broadcast matmul can
        # read each at an aligned base partition
        mask_sb = sb.tile([33, 49], F32R, tag="msk", bufs=2)
        lg = sb.tile([2, 49], F32, tag="lg", bufs=2)
        nc.scalar.activation(out=lg, in_=pt_l[0:2, :49], func=COPY)
        nc.vector.tensor_scalar(
            out=mask_sb[0:2, :], in0=lg, scalar1=w["dfc"], scalar2=0.0,
            op0=ADD, op1=ISGT)
        nc.sync.dma_start(out=mask_sb[32:33, :], in_=mask_sb[1:2, :])
        nc.sync.dma_start(out=mask_d[s], in_=mask_sb[0:2, :])
        # broadcast mask to 128 partitions: col-upsample on DVE, bounce the
        # tiny row through DRAM, DMA back with a partition-step-0 source
        mask_up = sb.tile([33, 196], F32R, tag="mskup", bufs=2)
        for g in range(2):
            nc.vector.tensor_copy(
                out=mask_up[32 * g:32 * g + 1, :].rearrange(
                    "p (a c r) -> p a c r", a=7, c=7),
                in_=mask_sb[32 * g:32 * g + 1, :].rearrange(
                    "p (a c) -> p a c", a=7)
                .unsqueeze(3).broadcast_to([1, 7, 7, 4]))
        scr = dramp.tile([2, 196], F32R)
        nc.sync.dma_start(out=scr[0:1, :], in_=mask_up[0:1, :])
        nc.sync.dma_start(out=scr[1:2, :], in_=mask_up[32:33, :])
        m28c = sb.tile([128, 2, 196], F32R, tag="m28", bufs=1)
        scr_ap = scr[:, :]
        bsrc = bass.AP(tensor=scr_ap.tensor, offset=scr_ap.offset,
                       ap=[[0, 128]] + [list(p) for p in scr_ap.ap])
        nc.sync.dma_start(out=m28c, in_=bsrc)

        # ================= refine branch (f32r) =================
        # conv1r grouped 1x1, t1 = mask * relu(bn1(conv))
        for g in range(2):
            pr = ps.tile([128, 2, 512], F32, tag="p2", bufs=3)
            for h in range(2):
                for kc in range(4):
                    nc.tensor.matmul(
                        pr[:, h, :392],
                        r32(w["w1r"][:, kc, g, :]),
                        r32(xp[:, 4 * g + kc, 1 + h * 14:15 + h * 14, 1:29]),
                        start=(kc == 0), stop=(kc == 3))
            tmp = sb.tile([128, 784], F32, tag="tmp", bufs=3)
            nc.scalar.activation(
                out=tmp.rearrange("p (h f) -> p h f", h=2),
                in_=pr[:, :, :392], func=RELU, bias=w["b1r"][:, g:g + 1],
                scale=1.0)
            nc.vector.tensor_tensor(
                out=t1_pad[:, g, 1:29, 1:29].rearrange(
                    "p (a r) b -> p a r b", r=4),
                in0=tmp.rearrange("p (a r b) -> p a r b", a=7, r=4),
                in1=m28c[:, g, :].rearrange("p (a c) -> p a c", a=7)
                    .unsqueeze(2).broadcast_to([128, 7, 4, 28]),
                op=MULT)

        # conv2r grouped 3x3, t2 = mask * relu(bn2(conv))
        t2 = sb.tile([128, 2, 784], F32R, tag="t2", bufs=1)
        for g in range(2):
            pr = ps.tile([128, 2, 512], F32, tag="p2", bufs=3)
            for h in range(2):
                for ti, (dy, dx) in enumerate(taps):
                    nc.tensor.matmul(
                        pr[:, h, :392],
                        r32(w["w2r"][:, g, ti, :]),
                        r32(t1_pad[:, g, h * 14 + dy:h * 14 + dy + 14,
                                   dx:dx + 28]),
                        start=(ti == 0), stop=(ti == 8))
            tmp = sb.tile([128, 784], F32, tag="tmp", bufs=3)
            nc.scalar.activation(
                out=tmp.rearrange("p (h f) -> p h f", h=2),
                in_=pr[:, :, :392], func=RELU, bias=w["b2r"][:, g:g + 1],
                scale=1.0)
            nc.vector.tensor_tensor(
                out=t2[:, g, :].rearrange("p (a r b) -> p a r b", a=7, r=4),
                in0=tmp.rearrange("p (a r b) -> p a r b", a=7, r=4),
                in1=m28c[:, g, :].rearrange("p (a c) -> p a c", a=7)
                    .unsqueeze(2).broadcast_to([128, 7, 4, 28]),
                op=MULT)

        # conv3r grouped 1x1 + final fuse, per output chunk
        for mc in range(NCH):
            g = mc // 4
            # s1 = x + upsample2(xb3) + b3r   (gpsimd, col-parity split)
            s1 = sb.tile([128, 784], F32, tag="s1", bufs=3)
            s14 = s1.rearrange("p (a b c d) -> p a b c d", a=14, b=2, c=14)
            xi4 = xp[:, mc, 1:29, 1:29].rearrange(
                "p (a b) (c d) -> p a b c d", b=2, d=2)
            up2d = xb3[:, mc, :].rearrange("p (a c) -> p a c", a=14)
            for rep in range(2):
                for par in range(2):
                    nc.vector.scalar_tensor_tensor(
                        out=s14[:, :, rep, :, par],
                        in0=xi4[:, :, rep, :, par],
                        scalar=w["b3r"][:, mc:mc + 1], in1=up2d,
                        op0=ADD, op1=ADD)
            pr = ps.tile([128, 2, 512], F32, tag="p2", bufs=3)
            for h in range(2):
                nc.tensor.matmul(
                    pr[:, h, :392],
                    r32(w["w3r"][:, g, (mc % 4) * 128:(mc % 4 + 1) * 128]),
                    r32(t2[:, g, :].rearrange("p (h2 f) -> p h2 f",
                                              h2=2)[:, h]),
                    start=True, stop=True)
            o_t = sb.tile([128, 784], F32, tag="ot", bufs=3)
            nc.vector.tensor_tensor(
                out=o_t.rearrange("p (h f) -> p h f", h=2),
                in0=pr[:, :, :392],
                in1=s1.rearrange("p (h f) -> p h f", h=2), op=ADD)
            o2 = sb.tile([128, 784], F32, tag="o2", bufs=3)
            nc.scalar.activation(out=o2, in_=o_t, func=RELU)
            nc.sync.dma_start(out=out_d[s, mc * 128:(mc + 1) * 128], in_=o2)

    dma_in(0)
    for s in range(SPC):
        if s + 1 < SPC:
            dma_in(s + 1)
        compute(s)


def kernel(**inputs):
    if "nc" not in _cache:
        _cache["nc"] = _build_nc()
    nc = _cache["nc"]
    wn = _prep_weights(**{k: v for k, v in inputs.items() if k != "x"})
    x = np.ascontiguousarray(np.asarray(inputs["x"], np.float32))

    core_ids = list(range(NCORES))
    in_maps = []
    for c in range(NCORES):
        m = {"x": np.ascontiguousarray(x[c * SPC:(c + 1) * SPC])}
        m.update(wn)
        in_maps.append(m)
    res = run_bass_kernel_spmd(nc, in_maps, core_ids)
    out = np.concatenate([res.results[c]["out"] for c in range(NCORES)], 0)
    mask = np.concatenate([res.results[c]["mask"] for c in range(NCORES)], 0)
    return out.astype(np.float32), mask.astype(np.float32)
